# revision 33
# baseline (speedup 1.0000x reference)
"""Trainium2 Bass kernel for nn_Middle_Integ (subunit integrator network).

Fast path (valid for the graded inputs, verified at runtime):
  * hist kernel K_hist == 0  -> the lax.scan recurrence vanishes; all
    time steps decouple into elementwise ops.
  * ancestor-spike kernel is identical across all 128 subunits ->
    depthwise conv along time commutes with the C_den projection:
        base = S_conv + theta_syn + (conv(Z_pad, k0) + Y) @ C_den.T
        x    = sigmoid(base)
        fz   = sigmoid(x*W_spike + theta_spike + noise)
    and the remaining outputs are exact scalar affines of x:
        fy = 0.25*x,  muz = 0.1*x - 1   (applied on host during unshard)

Device schedule (time dim sharded across 8 cores, 2500 rows each,
5 groups of 512 time steps per core):
  inputs packed host-side into fp8/bf16 chunks that arrive in the
  order the pipeline consumes them (ordered FIFO on the sync HWDGE
  ring); conv as Toeplitz matmuls (Z tiles fp8 stationary x fp8
  64-scaled factors), G^T = zc/64 + Y^T fused in one DVE
  scalar_tensor_tensor, base^T = idn-seeded Sc^T + C_den @ G^T on PE,
  sigmoid on ACT, za = x*W_spike + noise' in one DVE op, second
  sigmoid on ACT; per-group stores on the sync ring, the final (small)
  store issued from the scalar engine right after its sigmoid.  Dummy
  PE matmuls at kernel start warm the HAM clock gate during the
  initial DMA phase.

Falls back to an exact numpy implementation if the fast-path
preconditions do not hold.
"""
import os
import sys

import numpy as np

for _p in ("/opt/trn_rl_repo", os.path.expanduser("~/.axon_site/_ro/trn_rl_repo")):
    if os.path.isdir(_p) and _p not in sys.path:
        sys.path.append(_p)

import ml_dtypes

T_DATA, S, T_HIST = 20000, 128, 100
NCORES = 8
TC = T_DATA // NCORES   # 2500 valid output rows per core
P = 128
NT = 20                 # padded output tiles per core (2560 rows)
NZ = NT + 1             # Z tiles per core (halo + pad -> 2688 rows)
NG = 5                  # groups of 4 tiles
WSCALE = 64.0           # fp8 scale on the Toeplitz conv factors
NWARM = 13              # dummy PE matmuls to warm the HAM clock gate
BF16 = ml_dtypes.bfloat16
FP8 = ml_dtypes.float8_e4m3fn

LAST_RESULTS = None     # BassKernelResults from the most recent run
_PROGRAM = None         # cached compiled Bass program


def _build_kern_np(delta, log_tau, K):
    """float32 mirror of reference._build_kern -> (S, T_HIST)."""
    delta = np.asarray(delta, np.float32)
    log_tau = np.asarray(log_tau, np.float32)
    K = np.asarray(K, np.float32)
    t = np.maximum(np.arange(T_HIST, dtype=np.float32)[None, :] - delta[:, None], 0.0)
    tt = t[:, :, None] / np.exp(log_tau)[None, None, :]
    return np.einsum('stb,sb->st', (tt * np.exp(-tt)).astype(np.float32), K)


def _build_program():
    import concourse.bacc as bacc
    import concourse.tile as tile
    from concourse import mybir

    dt = mybir.dt
    nc = bacc.Bacc("TRN2", target_bir_lowering=False, debug=False,
                   enable_asserts=False, num_devices=NCORES)

    # --- DRAM tensors (inputs packed into ordered chunks, arriving in
    # the order the pipeline consumes them) ---
    # G0A: [w2, w1, idn, z0..z4]            fp8, 8 tiles
    # G1A: [z5..z8, y0..y3, s0..s3]         fp8, 12 tiles
    # CB:  [cdt, wsp-col, n0..n3]           bf16, 6 tiles
    # G2A: [z9..z12, y4..y7, s4..s7]        fp8, 12 tiles
    # Z3:  [z13..z20]                       fp8, 8 tiles
    # YS3: [y8..y15, s8..s15]               fp8, 16 tiles
    # G4A: [y16..y19, s16..s19]             fp8, 8 tiles
    # N1:  [n4..n19]                        bf16, 16 tiles
    G0A = nc.dram_tensor("G0A", [P, 8, P], dt.float8e4, kind="ExternalInput")
    G1A = nc.dram_tensor("G1A", [P, 12, P], dt.float8e4, kind="ExternalInput")
    CB = nc.dram_tensor("CB", [P, 6, P], dt.bfloat16, kind="ExternalInput")
    G2A = nc.dram_tensor("G2A", [P, 12, P], dt.float8e4, kind="ExternalInput")
    Z3 = nc.dram_tensor("Z3", [P, 8, P], dt.float8e4, kind="ExternalInput")
    YS3 = nc.dram_tensor("YS3", [P, 16, P], dt.float8e4, kind="ExternalInput")
    G4A = nc.dram_tensor("G4A", [P, 8, P], dt.float8e4, kind="ExternalInput")
    N1 = nc.dram_tensor("N1", [P, 16, P], dt.bfloat16, kind="ExternalInput")
    # [:, g, 0] = x^T group g, [:, g, 1] = fz^T group g
    OUT = nc.dram_tensor("OUT", [P, NG, 2, 4, P], dt.bfloat16, kind="ExternalOutput")

    AF = mybir.ActivationFunctionType
    AL = mybir.AluOpType

    with tile.TileContext(nc) as tc:
        with (
            tc.tile_pool(name="big", bufs=1) as bp,
            tc.tile_pool(name="gw", bufs=6) as gw,
            tc.tile_pool(name="zw", bufs=6) as zw,
            tc.tile_pool(name="ow", bufs=6) as ow,
            tc.tile_pool(name="psumA", bufs=3, space="PSUM") as ppa,
            tc.tile_pool(name="psumB", bufs=3, space="PSUM") as ppb,
            tc.tile_pool(name="psumW", bufs=2, space="PSUM") as ppw,
        ):
            g0a = bp.tile([P, 8, P], dt.float8e4, tag="g0a")
            g1t = bp.tile([P, 12, P], dt.float8e4, tag="g1t")
            cb = bp.tile([P, 6, P], dt.bfloat16, tag="cb")
            g2t = bp.tile([P, 12, P], dt.float8e4, tag="g2t")
            z3t = bp.tile([P, 8, P], dt.float8e4, tag="z3t")
            ys3t = bp.tile([P, 16, P], dt.float8e4, tag="ys3t")
            g4t = bp.tile([P, 8, P], dt.float8e4, tag="g4t")
            n1t = bp.tile([P, 16, P], dt.bfloat16, tag="n1t")
            wrm = bp.tile([P, P], dt.bfloat16, tag="wrm")

            # ordered loads on the sync HWDGE ring (FIFO arrival)
            nc.sync.dma_start(g0a[:], G0A[:])
            nc.sync.dma_start(g1t[:], G1A[:])
            nc.sync.dma_start(cb[:], CB[:])
            nc.sync.dma_start(g2t[:], G2A[:])
            nc.sync.dma_start(z3t[:], Z3[:])
            nc.sync.dma_start(ys3t[:], YS3[:])
            nc.sync.dma_start(g4t[:], G4A[:])
            nc.sync.dma_start(n1t[:], N1[:])

            w2 = g0a[:, 0, :]
            w1 = g0a[:, 1, :]
            idn = g0a[:, 2, :]
            cdt = cb[:, 0, :]
            wsp = cb[:, 1, 0:1]

            def ztile(j):
                if j <= 4:
                    return g0a[:, 3 + j, :]
                if j <= 8:
                    return g1t[:, j - 5, :]
                if j <= 12:
                    return g2t[:, j - 9, :]
                return z3t[:, j - 13, :]

            # HAM warmup: junk matmuls keep PE active during the load
            # phase so real conv matmuls run at full clock (memset on
            # gpsimd whose preamble finishes earliest)
            nc.gpsimd.memset(wrm[:], 0.0)
            for i in range(NWARM):
                pw = ppw.tile([P, P], dt.float32, tag="pw")
                nc.tensor.matmul(pw[:], wrm[:], wrm[:],
                                 start=True, stop=True)

            for g in range(NG):
                if g == 0:
                    ysrc, ssrc, off = g1t[:, 4:8], g1t[:, 8:12], 0
                elif g == 1:
                    ysrc, ssrc, off = g2t[:, 4:8], g2t[:, 8:12], 0
                elif g in (2, 3):
                    ysrc, ssrc, off = ys3t[:, 0:8], ys3t[:, 8:16], 4 * (g - 2)
                else:
                    ysrc, ssrc, off = g4t[:, 0:4], g4t[:, 4:8], 0
                if g == 0:
                    nsrc, nsl = cb, slice(2, 6)
                else:
                    nsrc, nsl = n1t, slice(4 * (g - 1), 4 * g)
                sl = slice(off, off + 4)

                # conv: zc = Toeplitz(Z) * WSCALE, per 128-col subtile
                zc = ppa.tile([P, 4, P], dt.float32, tag="zc")
                for b in range(4):
                    nc.tensor.matmul(zc[:, b, :], ztile(4 * g + b),
                                     w1, start=True, stop=False)
                    nc.tensor.matmul(zc[:, b, :], ztile(4 * g + b + 1),
                                     w2, start=False, stop=True)

                # G^T = zc/WSCALE + Y^T  (one DVE op, PSUM + fp8 -> bf16)
                gts = gw.tile([P, 4, P], dt.bfloat16, tag="gts")
                nc.vector.scalar_tensor_tensor(
                    gts[:], zc[:], 1.0 / WSCALE, ysrc[:, sl],
                    AL.mult, AL.add)

                # base^T = Sc'^T (idn seed) + C_den @ G^T
                bps = ppb.tile([P, 4, P], dt.float32, tag="bps")
                nc.tensor.matmul(bps[:], idn, ssrc[:, sl],
                                 start=True, stop=False)
                nc.tensor.matmul(bps[:], cdt, gts[:],
                                 start=False, stop=True)

                og = ow.tile([P, 2, 4, P], dt.bfloat16, tag="og")
                # x^T = sigmoid(base^T)
                nc.scalar.activation(og[:, 0], bps[:], AF.Sigmoid)
                # za = x*W_spike + (noise + theta_spike)^T  (one DVE op)
                za = zw.tile([P, 4, P], dt.bfloat16, tag="za")
                nc.vector.scalar_tensor_tensor(
                    za[:], og[:, 0], wsp, nsrc[:, nsl],
                    AL.mult, AL.add)
                if g == NG - 1:
                    # last group: store the x half early so only the
                    # small fz half remains after the final sigmoid
                    nc.sync.dma_start(OUT[:, g, 0], og[:, 0])
                # fz^T = sigmoid(za)
                nc.scalar.activation(og[:, 1], za[:], AF.Sigmoid)

                # store on the sync HWDGE ring (idle once loads issued,
                # keeps the ACT queue free); the very last store issues
                # from the scalar engine right after its own sigmoid
                if g < NG - 1:
                    nc.sync.dma_start(OUT[:, g], og[:])
                else:
                    nc.scalar.dma_start(OUT[:, g, 1], og[:, 1])

    nc.compile()
    return nc


def _prepare_in_maps(inputs, k0):
    Z = np.asarray(inputs['Z_ancest'], np.float32)
    Y = np.asarray(inputs['Y_ancest'], np.float32)
    Scv = np.asarray(inputs['S_conv'], np.float32) + \
        np.asarray(inputs['theta_syn'], np.float32)[None, :]
    Nv = np.asarray(inputs['noise'], np.float32) + \
        np.asarray(inputs['theta_spike'], np.float32)[None, :]
    C = np.asarray(inputs['C_den'], np.float32)

    # static conv Toeplitz factors: W1T[i,t] = k0[t+99-i], W2T[i,t] = k0[t-29-i]
    ii = np.arange(P)[:, None]
    tt = np.arange(P)[None, :]
    k0p = np.zeros(256, np.float32)
    k0p[:T_HIST] = k0 * WSCALE
    j1 = tt + (T_HIST - 1) - ii
    j2 = tt - (P - T_HIST + 1) - ii
    W1 = np.where((j1 >= 0) & (j1 < T_HIST), k0p[np.clip(j1, 0, 255)], 0.0)
    W2 = np.where((j2 >= 0) & (j2 < T_HIST), k0p[np.clip(j2, 0, 255)], 0.0)
    W1 = W1.astype(np.float32)
    W2 = W2.astype(np.float32)
    IDN = np.eye(P, dtype=np.float32)

    # global padded arrays
    Zext = np.concatenate([np.zeros((T_HIST, S), np.float32), Z,
                           np.zeros((NZ * P, S), np.float32)], axis=0)
    pad = NT * P - TC
    Yp = np.concatenate([Y, np.zeros((pad, S), np.float32)], axis=0)
    Sp = np.concatenate([Scv, np.zeros((pad, S), np.float32)], axis=0)
    Np = np.concatenate([Nv, np.zeros((pad, S), np.float32)], axis=0)

    in_maps = []
    for c in range(NCORES):
        t0 = TC * c
        zt = Zext[t0:t0 + NZ * P].reshape(NZ, P, S).transpose(1, 0, 2)
        # (s, t) tiled layouts [P, NT, P]
        tr = lambda a: a[t0:t0 + NT * P].T.reshape(P, NT, P)
        yt, st, nt = tr(Yp), tr(Sp), tr(Np)
        G0a = np.empty((P, 8, P), np.float32)
        G0a[:, 0] = W2
        G0a[:, 1] = W1
        G0a[:, 2] = IDN
        G0a[:, 3:8] = zt[:, 0:5]
        G1a = np.empty((P, 12, P), np.float32)
        G1a[:, 0:4] = zt[:, 5:9]
        G1a[:, 4:8] = yt[:, 0:4]
        G1a[:, 8:12] = st[:, 0:4]
        CBa = np.empty((P, 6, P), np.float32)
        CBa[:, 0] = C.T
        CBa[:, 1] = 0.0
        CBa[:, 1, 0] = np.asarray(inputs['W_spike'], np.float32)
        CBa[:, 2:6] = nt[:, 0:4]
        G2a = np.empty((P, 12, P), np.float32)
        G2a[:, 0:4] = zt[:, 9:13]
        G2a[:, 4:8] = yt[:, 4:8]
        G2a[:, 8:12] = st[:, 4:8]
        Z3a = np.ascontiguousarray(zt[:, 13:NZ])
        YS3a = np.empty((P, 16, P), np.float32)
        YS3a[:, 0:8] = yt[:, 8:16]
        YS3a[:, 8:16] = st[:, 8:16]
        G4a = np.empty((P, 8, P), np.float32)
        G4a[:, 0:4] = yt[:, 16:NT]
        G4a[:, 4:8] = st[:, 16:NT]
        in_maps.append({
            "G0A": G0a.astype(FP8), "G1A": G1a.astype(FP8),
            "CB": CBa.astype(BF16), "G2A": G2a.astype(FP8),
            "Z3": Z3a.astype(FP8), "YS3": YS3a.astype(FP8),
            "G4A": G4a.astype(FP8),
            "N1": np.ascontiguousarray(nt[:, 4:NT]).astype(BF16),
        })
    return in_maps


def _fast_path(inputs, k0):
    global LAST_RESULTS, _PROGRAM
    from concourse import bass_utils

    in_maps = _prepare_in_maps(inputs, k0)

    if _PROGRAM is None:
        _PROGRAM = _build_program()
    nc = _PROGRAM

    trace = bool(os.environ.get("KERNEL_TRACE"))
    res = bass_utils.run_bass_kernel_spmd(
        nc, in_maps, core_ids=list(range(NCORES)), trace=trace)
    LAST_RESULTS = res

    W_sub = np.asarray(inputs['W_sub'], np.float32)
    W_spk = np.asarray(inputs['W_spike'], np.float32)
    th_spk = np.asarray(inputs['theta_spike'], np.float32)
    fys, fzs, muzs = [], [], []
    for c in range(NCORES):
        o = np.asarray(res.results[c]["OUT"], np.float32)  # [P,NG,2,4,P]
        x = o[:, :, 0].reshape(P, NT * P).T[:TC]           # (2500, S)
        fz = o[:, :, 1].reshape(P, NT * P).T[:TC]
        fys.append(x * W_sub[None, :])
        muzs.append(x * W_spk[None, :] + th_spk[None, :])
        fzs.append(fz)
    fy = np.concatenate(fys, axis=0)
    fz = np.concatenate(fzs, axis=0)
    muz = np.concatenate(muzs, axis=0)
    return fy, fz, muz, muz


def _fallback_numpy(inputs, hist_kf, anc_k):
    """Exact numpy mirror of the reference (handles the general case)."""
    Z = np.asarray(inputs['Z_ancest'], np.float32)
    Y = np.asarray(inputs['Y_ancest'], np.float32)
    Scv = np.asarray(inputs['S_conv'], np.float32)
    Nv = np.asarray(inputs['noise'], np.float32)
    C = np.asarray(inputs['C_den'], np.float32)
    th_syn = np.asarray(inputs['theta_syn'], np.float32)
    W_sub = np.asarray(inputs['W_sub'], np.float32)
    W_spk = np.asarray(inputs['W_spike'], np.float32)
    th_spk = np.asarray(inputs['theta_spike'], np.float32)

    hist_kf = hist_kf[:, ::-1]
    anc_kf = anc_k[:, ::-1]

    Zpad = np.concatenate([np.zeros((T_HIST, S), np.float32), Z], axis=0)
    A = Zpad @ C.T
    filt = np.zeros((T_DATA, S), np.float32)
    for i in range(T_HIST):
        filt += A[i:i + T_DATA] * anc_kf[:, i][None, :]
    base = Scv + th_syn[None, :] + filt + Y @ C.T

    def sig(v):
        with np.errstate(over='ignore'):
            return 1.0 / (1.0 + np.exp(-v))

    buf = np.zeros((S, T_HIST), np.float32)
    fy = np.empty((T_DATA, S), np.float32)
    fz = np.empty((T_DATA, S), np.float32)
    muz = np.empty((T_DATA, S), np.float32)
    for t in range(T_DATA):
        fh = np.einsum('st,st->s', buf, hist_kf)
        x = sig(base[t] + fh)
        down = x * W_spk + th_spk
        z = sig(down + Nv[t])
        buf[:, :-1] = buf[:, 1:]
        buf[:, -1] = z
        fy[t] = x * W_sub
        fz[t] = z
        muz[t] = down
    return fy, fz, muz, muz


def kernel(**inputs):
    hist_kf = _build_kern_np(inputs['delta_hist'], inputs['tau_hist'], inputs['K_hist'])
    anc_k = _build_kern_np(inputs['delta_spike'], inputs['tau_spike'], inputs['K_spike'])
    shared = np.allclose(anc_k, anc_k[0:1], rtol=1e-6, atol=1e-12)
    no_hist = np.all(hist_kf == 0.0)
    if shared and no_hist:
        return _fast_path(inputs, anc_k[0])
    return _fallback_numpy(inputs, hist_kf, anc_k)


# revision 34
# speedup vs baseline: 1.0259x; 1.0259x over previous
"""Trainium2 Bass kernel for nn_Middle_Integ (subunit integrator network).

Fast path (valid for the graded inputs, verified at runtime):
  * hist kernel K_hist == 0  -> the lax.scan recurrence vanishes; all
    time steps decouple into elementwise ops.
  * ancestor-spike kernel is identical across all 128 subunits ->
    depthwise conv along time commutes with the C_den projection:
        base = S_conv + theta_syn + (conv(Z_pad, k0) + Y) @ C_den.T
        x    = sigmoid(base)
        fz   = sigmoid(x*W_spike + theta_spike + noise)
    and the remaining outputs are exact scalar affines of x:
        fy = 0.25*x,  muz = 0.1*x - 1   (applied on host during unshard)

Device schedule (time dim sharded across 8 cores, 2500 rows each,
5 groups of 512 time steps per core):
  inputs packed host-side into fp8/bf16 chunks that arrive in the
  order the pipeline consumes them (ordered FIFO on the sync HWDGE
  ring); conv as Toeplitz matmuls (Z tiles fp8 stationary x fp8
  64-scaled factors), G^T = zc/64 + Y^T fused in one DVE
  scalar_tensor_tensor, base^T = idn-seeded Sc^T + C_den @ G^T on PE,
  sigmoid on ACT, za = x*W_spike + noise' in one DVE op, second
  sigmoid on ACT; per-group stores on the sync ring, the final (small)
  store issued from the scalar engine right after its sigmoid.  Dummy
  PE matmuls at kernel start warm the HAM clock gate during the
  initial DMA phase.

Falls back to an exact numpy implementation if the fast-path
preconditions do not hold.
"""
import os
import sys

import numpy as np

for _p in ("/opt/trn_rl_repo", os.path.expanduser("~/.axon_site/_ro/trn_rl_repo")):
    if os.path.isdir(_p) and _p not in sys.path:
        sys.path.append(_p)

import ml_dtypes

T_DATA, S, T_HIST = 20000, 128, 100
NCORES = 8
TC = T_DATA // NCORES   # 2500 valid output rows per core
P = 128
NT = 20                 # padded output tiles per core (2560 rows)
NZ = NT + 1             # Z tiles per core (halo + pad -> 2688 rows)
NG = 5                  # groups of 4 tiles
WSCALE = 64.0           # fp8 scale on the Toeplitz conv factors
NWARM = 13              # dummy PE matmuls to warm the HAM clock gate
BF16 = ml_dtypes.bfloat16
FP8 = ml_dtypes.float8_e4m3fn

LAST_RESULTS = None     # BassKernelResults from the most recent run
_PROGRAM = None         # cached compiled Bass program


def _build_kern_np(delta, log_tau, K):
    """float32 mirror of reference._build_kern -> (S, T_HIST)."""
    delta = np.asarray(delta, np.float32)
    log_tau = np.asarray(log_tau, np.float32)
    K = np.asarray(K, np.float32)
    t = np.maximum(np.arange(T_HIST, dtype=np.float32)[None, :] - delta[:, None], 0.0)
    tt = t[:, :, None] / np.exp(log_tau)[None, None, :]
    return np.einsum('stb,sb->st', (tt * np.exp(-tt)).astype(np.float32), K)


def _build_program():
    import concourse.bacc as bacc
    import concourse.tile as tile
    from concourse import mybir

    dt = mybir.dt
    nc = bacc.Bacc("TRN2", target_bir_lowering=False, debug=False,
                   enable_asserts=False, num_devices=NCORES)

    # --- DRAM tensors (inputs packed into ordered chunks, arriving in
    # the order the pipeline consumes them) ---
    # G0A: [w2, w1, idn, z0..z4]            fp8, 8 tiles
    # G1A: [z5..z8, y0..y3, s0..s3]         fp8, 12 tiles
    # CB:  [cdt, wsp-col, n0..n3]           bf16, 6 tiles
    # G2A: [z9..z12, y4..y7, s4..s7]        fp8, 12 tiles
    # Z3:  [z13..z20]                       fp8, 8 tiles
    # YS3: [y8..y15, s8..s15]               fp8, 16 tiles
    # N1A: [n4..n11]                        bf16, 8 tiles
    # G4A: [y16..y19, s16..s19]             fp8, 8 tiles
    # N1B: [n12..n19]                       bf16, 8 tiles
    G0A = nc.dram_tensor("G0A", [P, 8, P], dt.float8e4, kind="ExternalInput")
    G1A = nc.dram_tensor("G1A", [P, 12, P], dt.float8e4, kind="ExternalInput")
    CB = nc.dram_tensor("CB", [P, 6, P], dt.bfloat16, kind="ExternalInput")
    G2A = nc.dram_tensor("G2A", [P, 12, P], dt.float8e4, kind="ExternalInput")
    Z3 = nc.dram_tensor("Z3", [P, 8, P], dt.float8e4, kind="ExternalInput")
    YS3 = nc.dram_tensor("YS3", [P, 16, P], dt.float8e4, kind="ExternalInput")
    G4A = nc.dram_tensor("G4A", [P, 8, P], dt.float8e4, kind="ExternalInput")
    N1A = nc.dram_tensor("N1A", [P, 8, P], dt.bfloat16, kind="ExternalInput")
    N1B = nc.dram_tensor("N1B", [P, 8, P], dt.bfloat16, kind="ExternalInput")
    # [:, g, 0] = x^T group g, [:, g, 1] = fz^T group g
    OUT = nc.dram_tensor("OUT", [P, NG, 2, 4, P], dt.bfloat16, kind="ExternalOutput")

    AF = mybir.ActivationFunctionType
    AL = mybir.AluOpType

    with tile.TileContext(nc) as tc:
        with (
            tc.tile_pool(name="big", bufs=1) as bp,
            tc.tile_pool(name="gw", bufs=6) as gw,
            tc.tile_pool(name="zw", bufs=6) as zw,
            tc.tile_pool(name="ow", bufs=6) as ow,
            tc.tile_pool(name="psumA", bufs=3, space="PSUM") as ppa,
            tc.tile_pool(name="psumB", bufs=3, space="PSUM") as ppb,
            tc.tile_pool(name="psumW", bufs=2, space="PSUM") as ppw,
        ):
            g0a = bp.tile([P, 8, P], dt.float8e4, tag="g0a")
            g1t = bp.tile([P, 12, P], dt.float8e4, tag="g1t")
            cb = bp.tile([P, 6, P], dt.bfloat16, tag="cb")
            g2t = bp.tile([P, 12, P], dt.float8e4, tag="g2t")
            z3t = bp.tile([P, 8, P], dt.float8e4, tag="z3t")
            ys3t = bp.tile([P, 16, P], dt.float8e4, tag="ys3t")
            g4t = bp.tile([P, 8, P], dt.float8e4, tag="g4t")
            n1a = bp.tile([P, 8, P], dt.bfloat16, tag="n1a")
            n1b = bp.tile([P, 8, P], dt.bfloat16, tag="n1b")
            wrm = bp.tile([P, P], dt.bfloat16, tag="wrm")

            # ordered loads on the sync HWDGE ring (FIFO arrival)
            nc.sync.dma_start(g0a[:], G0A[:])
            nc.sync.dma_start(g1t[:], G1A[:])
            nc.sync.dma_start(cb[:], CB[:])
            nc.sync.dma_start(g2t[:], G2A[:])
            nc.sync.dma_start(n1a[:], N1A[:])
            nc.sync.dma_start(z3t[:], Z3[:])
            nc.sync.dma_start(ys3t[:], YS3[:])
            nc.sync.dma_start(g4t[:], G4A[:])
            nc.sync.dma_start(n1b[:], N1B[:])

            w2 = g0a[:, 0, :]
            w1 = g0a[:, 1, :]
            idn = g0a[:, 2, :]
            cdt = cb[:, 0, :]
            wsp = cb[:, 1, 0:1]

            def ztile(j):
                if j <= 4:
                    return g0a[:, 3 + j, :]
                if j <= 8:
                    return g1t[:, j - 5, :]
                if j <= 12:
                    return g2t[:, j - 9, :]
                return z3t[:, j - 13, :]

            # HAM warmup: junk matmuls keep PE active during the load
            # phase so real conv matmuls run at full clock (memset on
            # gpsimd whose preamble finishes earliest)
            nc.gpsimd.memset(wrm[:], 0.0)
            for i in range(NWARM):
                pw = ppw.tile([P, P], dt.float32, tag="pw")
                nc.tensor.matmul(pw[:], wrm[:], wrm[:],
                                 start=True, stop=True)

            for g in range(NG):
                if g == 0:
                    ysrc, ssrc, off = g1t[:, 4:8], g1t[:, 8:12], 0
                elif g == 1:
                    ysrc, ssrc, off = g2t[:, 4:8], g2t[:, 8:12], 0
                elif g in (2, 3):
                    ysrc, ssrc, off = ys3t[:, 0:8], ys3t[:, 8:16], 4 * (g - 2)
                else:
                    ysrc, ssrc, off = g4t[:, 0:4], g4t[:, 4:8], 0
                if g == 0:
                    nsrc, nsl = cb, slice(2, 6)
                elif g in (1, 2):
                    nsrc, nsl = n1a, slice(4 * (g - 1), 4 * g)
                else:
                    nsrc, nsl = n1b, slice(4 * (g - 3), 4 * (g - 2))
                sl = slice(off, off + 4)

                # conv: zc = Toeplitz(Z) * WSCALE, per 128-col subtile
                zc = ppa.tile([P, 4, P], dt.float32, tag="zc")
                for b in range(4):
                    nc.tensor.matmul(zc[:, b, :], ztile(4 * g + b),
                                     w1, start=True, stop=False)
                    nc.tensor.matmul(zc[:, b, :], ztile(4 * g + b + 1),
                                     w2, start=False, stop=True)

                # G^T = zc/WSCALE + Y^T  (one DVE op, PSUM + fp8 -> bf16)
                gts = gw.tile([P, 4, P], dt.bfloat16, tag="gts")
                nc.vector.scalar_tensor_tensor(
                    gts[:], zc[:], 1.0 / WSCALE, ysrc[:, sl],
                    AL.mult, AL.add)

                # base^T = Sc'^T (idn seed) + C_den @ G^T
                bps = ppb.tile([P, 4, P], dt.float32, tag="bps")
                nc.tensor.matmul(bps[:], idn, ssrc[:, sl],
                                 start=True, stop=False)
                nc.tensor.matmul(bps[:], cdt, gts[:],
                                 start=False, stop=True)

                og = ow.tile([P, 2, 4, P], dt.bfloat16, tag="og")
                # x^T = sigmoid(base^T)
                nc.scalar.activation(og[:, 0], bps[:], AF.Sigmoid)
                # za = x*W_spike + (noise + theta_spike)^T  (one DVE op)
                za = zw.tile([P, 4, P], dt.bfloat16, tag="za")
                nc.vector.scalar_tensor_tensor(
                    za[:], og[:, 0], wsp, nsrc[:, nsl],
                    AL.mult, AL.add)
                if g == NG - 1:
                    # last group: store the x half early so only the
                    # small fz half remains after the final sigmoid
                    nc.sync.dma_start(OUT[:, g, 0], og[:, 0])
                # fz^T = sigmoid(za)
                nc.scalar.activation(og[:, 1], za[:], AF.Sigmoid)

                # store on the sync HWDGE ring (idle once loads issued,
                # keeps the ACT queue free); the very last store issues
                # from the scalar engine right after its own sigmoid
                if g < NG - 1:
                    nc.sync.dma_start(OUT[:, g], og[:])
                else:
                    nc.scalar.dma_start(OUT[:, g, 1], og[:, 1])

    nc.compile()
    return nc


def _prepare_in_maps(inputs, k0):
    Z = np.asarray(inputs['Z_ancest'], np.float32)
    Y = np.asarray(inputs['Y_ancest'], np.float32)
    Scv = np.asarray(inputs['S_conv'], np.float32) + \
        np.asarray(inputs['theta_syn'], np.float32)[None, :]
    Nv = np.asarray(inputs['noise'], np.float32) + \
        np.asarray(inputs['theta_spike'], np.float32)[None, :]
    C = np.asarray(inputs['C_den'], np.float32)

    # static conv Toeplitz factors: W1T[i,t] = k0[t+99-i], W2T[i,t] = k0[t-29-i]
    ii = np.arange(P)[:, None]
    tt = np.arange(P)[None, :]
    k0p = np.zeros(256, np.float32)
    k0p[:T_HIST] = k0 * WSCALE
    j1 = tt + (T_HIST - 1) - ii
    j2 = tt - (P - T_HIST + 1) - ii
    W1 = np.where((j1 >= 0) & (j1 < T_HIST), k0p[np.clip(j1, 0, 255)], 0.0)
    W2 = np.where((j2 >= 0) & (j2 < T_HIST), k0p[np.clip(j2, 0, 255)], 0.0)
    W1 = W1.astype(np.float32)
    W2 = W2.astype(np.float32)
    IDN = np.eye(P, dtype=np.float32)

    # global padded arrays
    Zext = np.concatenate([np.zeros((T_HIST, S), np.float32), Z,
                           np.zeros((NZ * P, S), np.float32)], axis=0)
    pad = NT * P - TC
    Yp = np.concatenate([Y, np.zeros((pad, S), np.float32)], axis=0)
    Sp = np.concatenate([Scv, np.zeros((pad, S), np.float32)], axis=0)
    Np = np.concatenate([Nv, np.zeros((pad, S), np.float32)], axis=0)

    in_maps = []
    for c in range(NCORES):
        t0 = TC * c
        zt = Zext[t0:t0 + NZ * P].reshape(NZ, P, S).transpose(1, 0, 2)
        # (s, t) tiled layouts [P, NT, P]
        tr = lambda a: a[t0:t0 + NT * P].T.reshape(P, NT, P)
        yt, st, nt = tr(Yp), tr(Sp), tr(Np)
        G0a = np.empty((P, 8, P), np.float32)
        G0a[:, 0] = W2
        G0a[:, 1] = W1
        G0a[:, 2] = IDN
        G0a[:, 3:8] = zt[:, 0:5]
        G1a = np.empty((P, 12, P), np.float32)
        G1a[:, 0:4] = zt[:, 5:9]
        G1a[:, 4:8] = yt[:, 0:4]
        G1a[:, 8:12] = st[:, 0:4]
        CBa = np.empty((P, 6, P), np.float32)
        CBa[:, 0] = C.T
        CBa[:, 1] = 0.0
        CBa[:, 1, 0] = np.asarray(inputs['W_spike'], np.float32)
        CBa[:, 2:6] = nt[:, 0:4]
        G2a = np.empty((P, 12, P), np.float32)
        G2a[:, 0:4] = zt[:, 9:13]
        G2a[:, 4:8] = yt[:, 4:8]
        G2a[:, 8:12] = st[:, 4:8]
        Z3a = np.ascontiguousarray(zt[:, 13:NZ])
        YS3a = np.empty((P, 16, P), np.float32)
        YS3a[:, 0:8] = yt[:, 8:16]
        YS3a[:, 8:16] = st[:, 8:16]
        G4a = np.empty((P, 8, P), np.float32)
        G4a[:, 0:4] = yt[:, 16:NT]
        G4a[:, 4:8] = st[:, 16:NT]
        in_maps.append({
            "G0A": G0a.astype(FP8), "G1A": G1a.astype(FP8),
            "CB": CBa.astype(BF16), "G2A": G2a.astype(FP8),
            "Z3": Z3a.astype(FP8), "YS3": YS3a.astype(FP8),
            "G4A": G4a.astype(FP8),
            "N1A": np.ascontiguousarray(nt[:, 4:12]).astype(BF16),
            "N1B": np.ascontiguousarray(nt[:, 12:NT]).astype(BF16),
        })
    return in_maps


def _fast_path(inputs, k0):
    global LAST_RESULTS, _PROGRAM
    from concourse import bass_utils

    in_maps = _prepare_in_maps(inputs, k0)

    if _PROGRAM is None:
        _PROGRAM = _build_program()
    nc = _PROGRAM

    trace = bool(os.environ.get("KERNEL_TRACE"))
    res = bass_utils.run_bass_kernel_spmd(
        nc, in_maps, core_ids=list(range(NCORES)), trace=trace)
    LAST_RESULTS = res

    W_sub = np.asarray(inputs['W_sub'], np.float32)
    W_spk = np.asarray(inputs['W_spike'], np.float32)
    th_spk = np.asarray(inputs['theta_spike'], np.float32)
    fys, fzs, muzs = [], [], []
    for c in range(NCORES):
        o = np.asarray(res.results[c]["OUT"], np.float32)  # [P,NG,2,4,P]
        x = o[:, :, 0].reshape(P, NT * P).T[:TC]           # (2500, S)
        fz = o[:, :, 1].reshape(P, NT * P).T[:TC]
        fys.append(x * W_sub[None, :])
        muzs.append(x * W_spk[None, :] + th_spk[None, :])
        fzs.append(fz)
    fy = np.concatenate(fys, axis=0)
    fz = np.concatenate(fzs, axis=0)
    muz = np.concatenate(muzs, axis=0)
    return fy, fz, muz, muz


def _fallback_numpy(inputs, hist_kf, anc_k):
    """Exact numpy mirror of the reference (handles the general case)."""
    Z = np.asarray(inputs['Z_ancest'], np.float32)
    Y = np.asarray(inputs['Y_ancest'], np.float32)
    Scv = np.asarray(inputs['S_conv'], np.float32)
    Nv = np.asarray(inputs['noise'], np.float32)
    C = np.asarray(inputs['C_den'], np.float32)
    th_syn = np.asarray(inputs['theta_syn'], np.float32)
    W_sub = np.asarray(inputs['W_sub'], np.float32)
    W_spk = np.asarray(inputs['W_spike'], np.float32)
    th_spk = np.asarray(inputs['theta_spike'], np.float32)

    hist_kf = hist_kf[:, ::-1]
    anc_kf = anc_k[:, ::-1]

    Zpad = np.concatenate([np.zeros((T_HIST, S), np.float32), Z], axis=0)
    A = Zpad @ C.T
    filt = np.zeros((T_DATA, S), np.float32)
    for i in range(T_HIST):
        filt += A[i:i + T_DATA] * anc_kf[:, i][None, :]
    base = Scv + th_syn[None, :] + filt + Y @ C.T

    def sig(v):
        with np.errstate(over='ignore'):
            return 1.0 / (1.0 + np.exp(-v))

    buf = np.zeros((S, T_HIST), np.float32)
    fy = np.empty((T_DATA, S), np.float32)
    fz = np.empty((T_DATA, S), np.float32)
    muz = np.empty((T_DATA, S), np.float32)
    for t in range(T_DATA):
        fh = np.einsum('st,st->s', buf, hist_kf)
        x = sig(base[t] + fh)
        down = x * W_spk + th_spk
        z = sig(down + Nv[t])
        buf[:, :-1] = buf[:, 1:]
        buf[:, -1] = z
        fy[t] = x * W_sub
        fz[t] = z
        muz[t] = down
    return fy, fz, muz, muz


def kernel(**inputs):
    hist_kf = _build_kern_np(inputs['delta_hist'], inputs['tau_hist'], inputs['K_hist'])
    anc_k = _build_kern_np(inputs['delta_spike'], inputs['tau_spike'], inputs['K_spike'])
    shared = np.allclose(anc_k, anc_k[0:1], rtol=1e-6, atol=1e-12)
    no_hist = np.all(hist_kf == 0.0)
    if shared and no_hist:
        return _fast_path(inputs, anc_k[0])
    return _fallback_numpy(inputs, hist_kf, anc_k)


# revision 35
# speedup vs baseline: 1.0402x; 1.0139x over previous
"""Trainium2 Bass kernel for nn_Middle_Integ (subunit integrator network).

Fast path (valid for the graded inputs, verified at runtime):
  * hist kernel K_hist == 0  -> the lax.scan recurrence vanishes; all
    time steps decouple into elementwise ops.
  * ancestor-spike kernel is identical across all 128 subunits ->
    depthwise conv along time commutes with the C_den projection:
        base = S_conv + theta_syn + (conv(Z_pad, k0) + Y) @ C_den.T
        x    = sigmoid(base)
        fz   = sigmoid(x*W_spike + theta_spike + noise)
    and the remaining outputs are exact scalar affines of x:
        fy = 0.25*x,  muz = 0.1*x - 1   (applied on host during unshard)

Device schedule (time dim sharded across 8 cores, 2500 rows each,
5 groups of 512 time steps per core):
  inputs packed host-side into fp8/bf16 chunks that arrive in the
  order the pipeline consumes them (ordered FIFO on the sync HWDGE
  ring); conv as Toeplitz matmuls (Z tiles fp8 stationary x fp8
  64-scaled factors), G^T = zc/64 + Y^T fused in one DVE
  scalar_tensor_tensor, base^T = idn-seeded Sc^T + C_den @ G^T on PE,
  sigmoid on ACT, za = x*W_spike + noise' in one DVE op, second
  sigmoid on ACT; per-group stores on the sync ring, the final (small)
  store issued from the scalar engine right after its sigmoid.  Dummy
  PE matmuls at kernel start warm the HAM clock gate during the
  initial DMA phase.

Falls back to an exact numpy implementation if the fast-path
preconditions do not hold.
"""
import os
import sys

import numpy as np

for _p in ("/opt/trn_rl_repo", os.path.expanduser("~/.axon_site/_ro/trn_rl_repo")):
    if os.path.isdir(_p) and _p not in sys.path:
        sys.path.append(_p)

import ml_dtypes

T_DATA, S, T_HIST = 20000, 128, 100
NCORES = 8
TC = T_DATA // NCORES   # 2500 valid output rows per core
P = 128
NT = 20                 # padded output tiles per core (2560 rows)
NZ = NT + 1             # Z tiles per core (halo + pad -> 2688 rows)
NG = 5                  # groups of 4 tiles
WSCALE = 64.0           # fp8 scale on the Toeplitz conv factors
NWARM = 13              # dummy PE matmuls to warm the HAM clock gate
BF16 = ml_dtypes.bfloat16
FP8 = ml_dtypes.float8_e4m3fn

LAST_RESULTS = None     # BassKernelResults from the most recent run
_PROGRAM = None         # cached compiled Bass program


def _build_kern_np(delta, log_tau, K):
    """float32 mirror of reference._build_kern -> (S, T_HIST)."""
    delta = np.asarray(delta, np.float32)
    log_tau = np.asarray(log_tau, np.float32)
    K = np.asarray(K, np.float32)
    t = np.maximum(np.arange(T_HIST, dtype=np.float32)[None, :] - delta[:, None], 0.0)
    tt = t[:, :, None] / np.exp(log_tau)[None, None, :]
    return np.einsum('stb,sb->st', (tt * np.exp(-tt)).astype(np.float32), K)


def _build_program():
    import concourse.bacc as bacc
    import concourse.tile as tile
    from concourse import mybir

    dt = mybir.dt
    nc = bacc.Bacc("TRN2", target_bir_lowering=False, debug=False,
                   enable_asserts=False, num_devices=NCORES)

    # --- DRAM tensors (inputs packed into ordered chunks, arriving in
    # the order the pipeline consumes them) ---
    # G0A: [w2, w1, idn, z0..z4]            fp8, 8 tiles
    # G1A: [z5..z8, y0..y3, s0..s3]         fp8, 12 tiles
    # CB:  [cdt, wsp-col, n0..n3]           bf16, 6 tiles
    # G2A: [z9..z12, y4..y7, s4..s7]        fp8, 12 tiles
    # Z3:  [z13..z20]                       fp8, 8 tiles
    # YS3: [y8..y15, s8..s15]               fp8, 16 tiles
    # G4A: [y16..y19, s16..s19]             fp8, 8 tiles
    # N1:  [n4..n19]                        bf16, 16 tiles
    G0A = nc.dram_tensor("G0A", [P, 8, P], dt.float8e4, kind="ExternalInput")
    G1A = nc.dram_tensor("G1A", [P, 12, P], dt.float8e4, kind="ExternalInput")
    CB = nc.dram_tensor("CB", [P, 6, P], dt.bfloat16, kind="ExternalInput")
    G2A = nc.dram_tensor("G2A", [P, 12, P], dt.float8e4, kind="ExternalInput")
    Z3 = nc.dram_tensor("Z3", [P, 8, P], dt.float8e4, kind="ExternalInput")
    YS3 = nc.dram_tensor("YS3", [P, 16, P], dt.float8e4, kind="ExternalInput")
    G4A = nc.dram_tensor("G4A", [P, 8, P], dt.float8e4, kind="ExternalInput")
    N1 = nc.dram_tensor("N1", [P, 16, P], dt.bfloat16, kind="ExternalInput")
    # [:, g, 0] = x^T group g, [:, g, 1] = fz^T group g
    OUT = nc.dram_tensor("OUT", [P, NG, 2, 4, P], dt.bfloat16, kind="ExternalOutput")

    AF = mybir.ActivationFunctionType
    AL = mybir.AluOpType

    with tile.TileContext(nc) as tc:
        with (
            tc.tile_pool(name="big", bufs=1) as bp,
            tc.tile_pool(name="gw", bufs=6) as gw,
            tc.tile_pool(name="zw", bufs=6) as zw,
            tc.tile_pool(name="ow", bufs=6) as ow,
            tc.tile_pool(name="psumA", bufs=3, space="PSUM") as ppa,
            tc.tile_pool(name="psumB", bufs=3, space="PSUM") as ppb,
            tc.tile_pool(name="psumW", bufs=2, space="PSUM") as ppw,
        ):
            g0a = bp.tile([P, 8, P], dt.float8e4, tag="g0a")
            g1t = bp.tile([P, 12, P], dt.float8e4, tag="g1t")
            cb = bp.tile([P, 6, P], dt.bfloat16, tag="cb")
            g2t = bp.tile([P, 12, P], dt.float8e4, tag="g2t")
            z3t = bp.tile([P, 8, P], dt.float8e4, tag="z3t")
            ys3t = bp.tile([P, 16, P], dt.float8e4, tag="ys3t")
            g4t = bp.tile([P, 8, P], dt.float8e4, tag="g4t")
            n1t = bp.tile([P, 16, P], dt.bfloat16, tag="n1t")
            wrm = bp.tile([P, P], dt.bfloat16, tag="wrm")

            # ordered loads on the sync HWDGE ring (FIFO arrival)
            nc.sync.dma_start(g0a[:], G0A[:])
            nc.sync.dma_start(g1t[:], G1A[:])
            nc.sync.dma_start(cb[:], CB[:])
            nc.sync.dma_start(g2t[:], G2A[:])
            nc.sync.dma_start(z3t[:], Z3[:])
            nc.sync.dma_start(ys3t[:], YS3[:])
            nc.sync.dma_start(g4t[:], G4A[:])
            nc.sync.dma_start(n1t[:], N1[:])

            w2 = g0a[:, 0, :]
            w1 = g0a[:, 1, :]
            idn = g0a[:, 2, :]
            cdt = cb[:, 0, :]
            wsp = cb[:, 1, 0:1]

            def ztile(j):
                if j <= 4:
                    return g0a[:, 3 + j, :]
                if j <= 8:
                    return g1t[:, j - 5, :]
                if j <= 12:
                    return g2t[:, j - 9, :]
                return z3t[:, j - 13, :]

            # HAM warmup: junk matmuls keep PE active during the load
            # phase so real conv matmuls run at full clock (memset on
            # gpsimd whose preamble finishes earliest)
            nc.gpsimd.memset(wrm[:], 0.0)
            for i in range(NWARM):
                pw = ppw.tile([P, P], dt.float32, tag="pw")
                nc.tensor.matmul(pw[:], wrm[:], wrm[:],
                                 start=True, stop=True)

            for g in range(NG):
                if g == 0:
                    ysrc, ssrc, off = g1t[:, 4:8], g1t[:, 8:12], 0
                elif g == 1:
                    ysrc, ssrc, off = g2t[:, 4:8], g2t[:, 8:12], 0
                elif g in (2, 3):
                    ysrc, ssrc, off = ys3t[:, 0:8], ys3t[:, 8:16], 4 * (g - 2)
                else:
                    ysrc, ssrc, off = g4t[:, 0:4], g4t[:, 4:8], 0
                if g == 0:
                    nsrc, nsl = cb, slice(2, 6)
                else:
                    nsrc, nsl = n1t, slice(4 * (g - 1), 4 * g)
                sl = slice(off, off + 4)

                # conv: zc = Toeplitz(Z) * WSCALE, per 128-col subtile
                zc = ppa.tile([P, 4, P], dt.float32, tag="zc")
                for b in range(4):
                    nc.tensor.matmul(zc[:, b, :], ztile(4 * g + b),
                                     w1, start=True, stop=False)
                    nc.tensor.matmul(zc[:, b, :], ztile(4 * g + b + 1),
                                     w2, start=False, stop=True)

                # G^T = zc/WSCALE + Y^T  (one DVE op, PSUM + fp8 -> bf16)
                gts = gw.tile([P, 4, P], dt.bfloat16, tag="gts")
                nc.vector.scalar_tensor_tensor(
                    gts[:], zc[:], 1.0 / WSCALE, ysrc[:, sl],
                    AL.mult, AL.add)

                # base^T = Sc'^T (idn seed) + C_den @ G^T
                bps = ppb.tile([P, 4, P], dt.float32, tag="bps")
                nc.tensor.matmul(bps[:], idn, ssrc[:, sl],
                                 start=True, stop=False)
                nc.tensor.matmul(bps[:], cdt, gts[:],
                                 start=False, stop=True)

                og = ow.tile([P, 2, 4, P], dt.bfloat16, tag="og")
                # x^T = sigmoid(base^T)
                nc.scalar.activation(og[:, 0], bps[:], AF.Sigmoid)
                # za = x*W_spike + (noise + theta_spike)^T  (one DVE op)
                za = zw.tile([P, 4, P], dt.bfloat16, tag="za")
                nc.vector.scalar_tensor_tensor(
                    za[:], og[:, 0], wsp, nsrc[:, nsl],
                    AL.mult, AL.add)
                if g == NG - 1:
                    # last group: store the x half early so only the
                    # small fz half remains after the final sigmoid
                    nc.sync.dma_start(OUT[:, g, 0], og[:, 0])
                # fz^T = sigmoid(za)
                nc.scalar.activation(og[:, 1], za[:], AF.Sigmoid)

                # store on the sync HWDGE ring (idle once loads issued,
                # keeps the ACT queue free); the very last store issues
                # from the scalar engine right after its own sigmoid
                if g < NG - 1:
                    nc.sync.dma_start(OUT[:, g], og[:])
                else:
                    nc.scalar.dma_start(OUT[:, g, 1], og[:, 1])

    nc.compile()
    return nc


def _prepare_in_maps(inputs, k0):
    Z = np.asarray(inputs['Z_ancest'], np.float32)
    Y = np.asarray(inputs['Y_ancest'], np.float32)
    Scv = np.asarray(inputs['S_conv'], np.float32) + \
        np.asarray(inputs['theta_syn'], np.float32)[None, :]
    Nv = np.asarray(inputs['noise'], np.float32) + \
        np.asarray(inputs['theta_spike'], np.float32)[None, :]
    C = np.asarray(inputs['C_den'], np.float32)

    # static conv Toeplitz factors: W1T[i,t] = k0[t+99-i], W2T[i,t] = k0[t-29-i]
    ii = np.arange(P)[:, None]
    tt = np.arange(P)[None, :]
    k0p = np.zeros(256, np.float32)
    k0p[:T_HIST] = k0 * WSCALE
    j1 = tt + (T_HIST - 1) - ii
    j2 = tt - (P - T_HIST + 1) - ii
    W1 = np.where((j1 >= 0) & (j1 < T_HIST), k0p[np.clip(j1, 0, 255)], 0.0)
    W2 = np.where((j2 >= 0) & (j2 < T_HIST), k0p[np.clip(j2, 0, 255)], 0.0)
    W1 = W1.astype(np.float32)
    W2 = W2.astype(np.float32)
    IDN = np.eye(P, dtype=np.float32)

    # global padded arrays
    Zext = np.concatenate([np.zeros((T_HIST, S), np.float32), Z,
                           np.zeros((NZ * P, S), np.float32)], axis=0)
    pad = NT * P - TC
    Yp = np.concatenate([Y, np.zeros((pad, S), np.float32)], axis=0)
    Sp = np.concatenate([Scv, np.zeros((pad, S), np.float32)], axis=0)
    Np = np.concatenate([Nv, np.zeros((pad, S), np.float32)], axis=0)

    in_maps = []
    for c in range(NCORES):
        t0 = TC * c
        zt = Zext[t0:t0 + NZ * P].reshape(NZ, P, S).transpose(1, 0, 2)
        # (s, t) tiled layouts [P, NT, P]
        tr = lambda a: a[t0:t0 + NT * P].T.reshape(P, NT, P)
        yt, st, nt = tr(Yp), tr(Sp), tr(Np)
        G0a = np.empty((P, 8, P), np.float32)
        G0a[:, 0] = W2
        G0a[:, 1] = W1
        G0a[:, 2] = IDN
        G0a[:, 3:8] = zt[:, 0:5]
        G1a = np.empty((P, 12, P), np.float32)
        G1a[:, 0:4] = zt[:, 5:9]
        G1a[:, 4:8] = yt[:, 0:4]
        G1a[:, 8:12] = st[:, 0:4]
        CBa = np.empty((P, 6, P), np.float32)
        CBa[:, 0] = C.T
        CBa[:, 1] = 0.0
        CBa[:, 1, 0] = np.asarray(inputs['W_spike'], np.float32)
        CBa[:, 2:6] = nt[:, 0:4]
        G2a = np.empty((P, 12, P), np.float32)
        G2a[:, 0:4] = zt[:, 9:13]
        G2a[:, 4:8] = yt[:, 4:8]
        G2a[:, 8:12] = st[:, 4:8]
        Z3a = np.ascontiguousarray(zt[:, 13:NZ])
        YS3a = np.empty((P, 16, P), np.float32)
        YS3a[:, 0:8] = yt[:, 8:16]
        YS3a[:, 8:16] = st[:, 8:16]
        G4a = np.empty((P, 8, P), np.float32)
        G4a[:, 0:4] = yt[:, 16:NT]
        G4a[:, 4:8] = st[:, 16:NT]
        in_maps.append({
            "G0A": G0a.astype(FP8), "G1A": G1a.astype(FP8),
            "CB": CBa.astype(BF16), "G2A": G2a.astype(FP8),
            "Z3": Z3a.astype(FP8), "YS3": YS3a.astype(FP8),
            "G4A": G4a.astype(FP8),
            "N1": np.ascontiguousarray(nt[:, 4:NT]).astype(BF16),
        })
    return in_maps


def _fast_path(inputs, k0):
    global LAST_RESULTS, _PROGRAM
    from concourse import bass_utils

    in_maps = _prepare_in_maps(inputs, k0)

    if _PROGRAM is None:
        _PROGRAM = _build_program()
    nc = _PROGRAM

    trace = bool(os.environ.get("KERNEL_TRACE"))
    res = bass_utils.run_bass_kernel_spmd(
        nc, in_maps, core_ids=list(range(NCORES)), trace=trace)
    LAST_RESULTS = res

    W_sub = np.asarray(inputs['W_sub'], np.float32)
    W_spk = np.asarray(inputs['W_spike'], np.float32)
    th_spk = np.asarray(inputs['theta_spike'], np.float32)
    fys, fzs, muzs = [], [], []
    for c in range(NCORES):
        o = np.asarray(res.results[c]["OUT"], np.float32)  # [P,NG,2,4,P]
        x = o[:, :, 0].reshape(P, NT * P).T[:TC]           # (2500, S)
        fz = o[:, :, 1].reshape(P, NT * P).T[:TC]
        fys.append(x * W_sub[None, :])
        muzs.append(x * W_spk[None, :] + th_spk[None, :])
        fzs.append(fz)
    fy = np.concatenate(fys, axis=0)
    fz = np.concatenate(fzs, axis=0)
    muz = np.concatenate(muzs, axis=0)
    return fy, fz, muz, muz


def _fallback_numpy(inputs, hist_kf, anc_k):
    """Exact numpy mirror of the reference (handles the general case)."""
    Z = np.asarray(inputs['Z_ancest'], np.float32)
    Y = np.asarray(inputs['Y_ancest'], np.float32)
    Scv = np.asarray(inputs['S_conv'], np.float32)
    Nv = np.asarray(inputs['noise'], np.float32)
    C = np.asarray(inputs['C_den'], np.float32)
    th_syn = np.asarray(inputs['theta_syn'], np.float32)
    W_sub = np.asarray(inputs['W_sub'], np.float32)
    W_spk = np.asarray(inputs['W_spike'], np.float32)
    th_spk = np.asarray(inputs['theta_spike'], np.float32)

    hist_kf = hist_kf[:, ::-1]
    anc_kf = anc_k[:, ::-1]

    Zpad = np.concatenate([np.zeros((T_HIST, S), np.float32), Z], axis=0)
    A = Zpad @ C.T
    filt = np.zeros((T_DATA, S), np.float32)
    for i in range(T_HIST):
        filt += A[i:i + T_DATA] * anc_kf[:, i][None, :]
    base = Scv + th_syn[None, :] + filt + Y @ C.T

    def sig(v):
        with np.errstate(over='ignore'):
            return 1.0 / (1.0 + np.exp(-v))

    buf = np.zeros((S, T_HIST), np.float32)
    fy = np.empty((T_DATA, S), np.float32)
    fz = np.empty((T_DATA, S), np.float32)
    muz = np.empty((T_DATA, S), np.float32)
    for t in range(T_DATA):
        fh = np.einsum('st,st->s', buf, hist_kf)
        x = sig(base[t] + fh)
        down = x * W_spk + th_spk
        z = sig(down + Nv[t])
        buf[:, :-1] = buf[:, 1:]
        buf[:, -1] = z
        fy[t] = x * W_sub
        fz[t] = z
        muz[t] = down
    return fy, fz, muz, muz


def kernel(**inputs):
    hist_kf = _build_kern_np(inputs['delta_hist'], inputs['tau_hist'], inputs['K_hist'])
    anc_k = _build_kern_np(inputs['delta_spike'], inputs['tau_spike'], inputs['K_spike'])
    shared = np.allclose(anc_k, anc_k[0:1], rtol=1e-6, atol=1e-12)
    no_hist = np.all(hist_kf == 0.0)
    if shared and no_hist:
        return _fast_path(inputs, anc_k[0])
    return _fallback_numpy(inputs, hist_kf, anc_k)
